# revision 21
# baseline (speedup 1.0000x reference)
"""Causal self-attention TRN2 kernel.

B=4, T=2048, C=1024, H=16 heads, D=64. 8 NeuronCores: core = b*2 + g
(b = batch 0..3, g = head-group 0..1, 8 heads each). Tensor-parallel over
heads within a batch; host sums the two partial proj outputs per batch.

Device-side layout (per core, bf16 matmuls / fp32 PSUM):
  xT   [C, T]      x[b].T
  wqT  [C, 512]    qkv_w q-rows for this head group, transposed
  wkT  [C, 512]
  wvT  [C, 520]    v-rows transposed, augmented: per head 64 v-cols + 1
                   zero-col whose bias is 1.0 -> denominator lands on PSUM
                   partition 64 during the P@V matmul.
  bq   [512, 1], bk [512, 1], bv [1, 520]
  tri  [128, 128]  single causal triangle mask (tri[j,i]=1 iff j<=i)
  pwT  [512, C]    proj_w[:, g*512:(g+1)*512].T
  out yT [C, T]    bf16 partial output, transposed

Schedule: ci-major (query-chunk outer) software pipeline. Per (head,
chunk): S^T j-tiles computed in PAIRS into [128,1024] 2-bank PSUM tiles;
ONE exp per pair on ScalarE (amortizes ACT access latency); diag pairs
ordered (r2,r0),(r3,r1) to minimize the stale middle gap. P@V of group
g-2 interleaved between pairs (issued right after each pair's exp so the
exp gets on the ACT queue asap). The ones column gives the softmax
denominator on PSUM partition 64; all groups use the same DMA-repack
reciprocal path ([128,8]-shaped recip; a [1,512]-shaped recip costs
3.3us). QKV/vaug fill ci 0-2; ALL proj fills ci=3 (its ACT-bound groups
need the most PE filler); PV lag drops to 1 late so the tail drains
early.
"""

import numpy as np
import ml_dtypes

B, T, C = 4, 2048, 1024
H, D = 16, 64
HPC = 8          # heads per core
P = 128
TCH = 512        # i-chunk (query) size
NCH = T // TCH   # 4
NJT = T // P     # 16 key tiles
CT = C // P      # 8 contraction tiles
VW = HPC * (D + 1)  # 520: augmented v width (ones col last per head)
HLF = VW // 2    # 260

QK_FP8 = True    # q/k projections via single-pass fp8e4m3 DoubleRow
WS = 16.0        # fp8 weight prescale (clears e4m3 subnormal cliff)

_CACHE = {}


def _build_nc():
    import concourse.bass as bass
    import concourse.mybir as mybir
    import concourse.tile as tile
    from concourse import bacc
    from contextlib import ExitStack

    bf16 = mybir.dt.bfloat16
    fp32 = mybir.dt.float32
    AF = mybir.ActivationFunctionType

    fp8 = mybir.dt.float8e4
    DRM = mybir.MatmulPerfMode.DoubleRow

    nc = bacc.Bacc()
    xT_d = nc.dram_tensor("xT", [C, T], bf16, kind="ExternalInput")
    if QK_FP8:
        xh_d = nc.dram_tensor("xh", [P, 4, 2, T], fp8, kind="ExternalInput")
        wq_d = nc.dram_tensor("wqh", [P, 4, 2, 512], fp8, kind="ExternalInput")
        wk_d = nc.dram_tensor("wkh", [P, 4, 2, 512], fp8, kind="ExternalInput")
    else:
        wq_d = nc.dram_tensor("wqT", [C, 512], bf16, kind="ExternalInput")
        wk_d = nc.dram_tensor("wkT", [C, 512], bf16, kind="ExternalInput")
    wv_d = nc.dram_tensor("wvT", [C, VW], bf16, kind="ExternalInput")
    bq_d = nc.dram_tensor("bq", [512, 1], fp32, kind="ExternalInput")
    bk_d = nc.dram_tensor("bk", [512, 1], fp32, kind="ExternalInput")
    bv_d = nc.dram_tensor("bv", [1, VW], bf16, kind="ExternalInput")
    tri_d = nc.dram_tensor("tri", [P, P], bf16, kind="ExternalInput")
    pw_d = nc.dram_tensor("pwT", [512, C], bf16, kind="ExternalInput")
    yT_d = nc.dram_tensor("yT", [C, T], bf16, kind="ExternalOutput")
    den_dram = nc.dram_tensor("den_scratch", [8, T], bf16)
    den2_dram = nc.dram_tensor("den2_scratch", [8, T], bf16)

    def bcast_part(ap, n):
        # replicate a [1, F] AP across n partitions (step-0 partition dim)
        return bass.AP(tensor=ap.tensor, offset=ap.offset,
                       ap=[[0, n]] + list(ap.ap[1:]))

    with ExitStack() as ctx:
        tc = ctx.enter_context(tile.TileContext(nc))
        consts = ctx.enter_context(tc.tile_pool(name="consts", bufs=1))
        xt_p = ctx.enter_context(tc.tile_pool(name="xt", bufs=1))
        vaug_p = ctx.enter_context(tc.tile_pool(name="vaug", bufs=1))
        qk_p = ctx.enter_context(tc.tile_pool(name="qk", bufs=1))
        pt_p = ctx.enter_context(tc.tile_pool(name="pt", bufs=20))
        ot_p = ctx.enter_context(tc.tile_pool(name="ot", bufs=1))
        st_p = ctx.enter_context(tc.tile_pool(name="st", bufs=4))
        rc_p = ctx.enter_context(tc.tile_pool(name="rc", bufs=2))
        rr_p = ctx.enter_context(tc.tile_pool(name="rr", bufs=2))
        ys_p = ctx.enter_context(tc.tile_pool(name="ys", bufs=3))
        ps_p = ctx.enter_context(tc.tile_pool(name="ps", bufs=1, space="PSUM"))

        # ---- DMA loads: x on sync, wq on scalar, wk on gpsimd (parallel
        # so the first k chains never head-block), rest later ----
        xt = xt_p.tile([P, CT, T], bf16, tag="xt")
        if QK_FP8:
            xh_t = xt_p.tile([P, 4, 2, T], fp8, tag="xh")
            wq_t = consts.tile([P, 4, 2, 512], fp8, tag="wq")
            wk_t = consts.tile([P, 4, 2, 512], fp8, tag="wk")
        else:
            wq_t = consts.tile([P, CT, 512], bf16, tag="wq")
            wk_t = consts.tile([P, CT, 512], bf16, tag="wk")
        wv_t = consts.tile([P, CT, VW], bf16, tag="wv")
        pw_t = consts.tile([P, 4, C], bf16, tag="pw")
        bq_t = consts.tile([P, 4, 1], fp32, tag="bq")
        bk_t = consts.tile([P, 4, 1], fp32, tag="bk")
        bv_t = consts.tile([P, VW], bf16, tag="bv")
        tri_t = consts.tile([P, P], bf16, tag="tri")

        def load_cols(eng, lo, hi, pieces):
            cper = CT // pieces
            for chn in range(pieces):
                eng.dma_start(
                    out=xt[:, cper * chn:cper * (chn + 1), lo:hi],
                    in_=xT_d[cper * chn * P:cper * (chn + 1) * P,
                             lo:hi].rearrange("(c p) f -> p c f", p=P))

        if QK_FP8:
            # critical-first, finely split so many DMA engines overlap:
            # first q chain needs wq c2-pieces + xh chunk-0 pieces
            for c2 in range(4):
                nc.scalar.dma_start(out=wq_t[:, c2, :, :], in_=wq_d[:, c2, :, :])
            nc.scalar.dma_start(out=bq_t,
                                in_=bq_d.ap().rearrange("(a p) o -> p a o", p=P))
            for c2 in range(4):
                nc.gpsimd.dma_start(out=wk_t[:, c2, :, :], in_=wk_d[:, c2, :, :])
            nc.gpsimd.dma_start(out=bk_t,
                                in_=bk_d.ap().rearrange("(a p) o -> p a o", p=P))
            for chn in range(4):
                nc.scalar.dma_start(out=xh_t[:, chn, :, 0:TCH],
                                    in_=xh_d[:, chn, :, 0:TCH])
            load_cols(nc.sync, 0, TCH, 8)
            for chn in range(4):
                nc.scalar.dma_start(out=xh_t[:, chn, :, TCH:2 * TCH],
                                    in_=xh_d[:, chn, :, TCH:2 * TCH])
            for c in range(0, CT, 2):
                nc.gpsimd.dma_start(
                    out=wv_t[:, c:c + 2, :],
                    in_=wv_d[c * P:(c + 2) * P, :].rearrange(
                        "(c p) f -> p c f", p=P))
            nc.gpsimd.dma_start(out=bv_t, in_=bcast_part(bv_d[0:1, :], P))
            nc.gpsimd.dma_start(out=tri_t, in_=tri_d[:, :])
            load_cols(nc.sync, TCH, 2 * TCH, 4)
            nc.scalar.dma_start(out=xh_t[:, :, :, 2 * TCH:4 * TCH],
                                in_=xh_d[:, :, :, 2 * TCH:4 * TCH])
            load_cols(nc.sync, 2 * TCH, 3 * TCH, 2)
            for m in range(4):
                nc.gpsimd.dma_start(
                    out=pw_t[:, m, :],
                    in_=pw_d[m * P:(m + 1) * P, :])
            load_cols(nc.sync, 3 * TCH, 4 * TCH, 2)
        else:
            for c in range(CT):
                nc.scalar.dma_start(out=wq_t[:, c, :], in_=wq_d[c * P:(c + 1) * P, :])
            nc.scalar.dma_start(out=bq_t,
                                in_=bq_d.ap().rearrange("(a p) o -> p a o", p=P))
            for c in range(CT):
                nc.gpsimd.dma_start(out=wk_t[:, c, :], in_=wk_d[c * P:(c + 1) * P, :])
            nc.gpsimd.dma_start(out=bk_t,
                                in_=bk_d.ap().rearrange("(a p) o -> p a o", p=P))
            load_cols(nc.sync, 0, TCH, 8)
            nc.gpsimd.dma_start(out=wv_t,
                                in_=wv_d.ap().rearrange("(c p) f -> p c f", p=P))
            nc.gpsimd.dma_start(out=bv_t, in_=bcast_part(bv_d[0:1, :], P))
            nc.gpsimd.dma_start(out=tri_t, in_=tri_d[:, :])
            load_cols(nc.sync, TCH, 2 * TCH, 4)
            nc.scalar.dma_start(out=pw_t,
                                in_=pw_d.ap().rearrange("(m p) f -> p m f", p=P))
            load_cols(nc.sync, 2 * TCH, 3 * TCH, 2)
            load_cols(nc.sync, 3 * TCH, 4 * TCH, 2)

        # ---- persistent tiles ----
        qts = [qk_p.tile([P, T], bf16, tag=f"qt{pr}", name=f"qt{pr}")
               for pr in range(4)]
        kts = [qk_p.tile([P, T], bf16, tag=f"kt{pr}", name=f"kt{pr}")
               for pr in range(4)]
        vaug = vaug_p.tile([P, NJT, VW], bf16, tag="vaug")
        ot = ot_p.tile([P, 4, T], bf16, tag="ot")
        ones_t = consts.tile([65, 64], bf16, tag="ones")
        nc.vector.memset(ones_t[64:65, :], 1.0)
        rr_tiles = {}

        def q_half(pr, tch, w_t, dst, b_t):
            ps = ps_p.tile([P, TCH], fp32, tag="mm", bufs=2,
                           name=f"pq{pr}_{tch}")
            if QK_FP8:
                # 4 DoubleRow steps of 256-contraction; psum = 16*(q|k)
                for c2 in range(4):
                    nc.tensor.matmul(ps, w_t[:, c2, :, pr * P:(pr + 1) * P],
                                     xh_t[:, c2, :, tch * TCH:(tch + 1) * TCH],
                                     start=(c2 == 0), stop=(c2 == 3),
                                     perf_mode=DRM)
            else:
                for c in range(CT):
                    rx = xt[:, c, tch * TCH:(tch + 1) * TCH]
                    nc.tensor.matmul(ps, w_t[:, c, pr * P:(pr + 1) * P], rx,
                                     start=(c == 0), stop=(c == CT - 1))
            nc.vector.tensor_scalar_add(dst[:, tch * TCH:(tch + 1) * TCH],
                                        ps, b_t[:, pr, :])

        def qk_chunk(pr, tch):
            q_half(pr, tch, wq_t, qts[pr], bq_t)
            q_half(pr, tch, wk_t, kts[pr], bk_t)

        def vaug_jt(jt):
            # sequential halves: tag-"mm" (bufs=2) chains overlap drains
            ps0 = ps_p.tile([P, HLF], fp32, tag="mm", bufs=2, name=f"v0_{jt}")
            for c in range(CT):
                nc.tensor.matmul(ps0, xt[:, c, jt * P:(jt + 1) * P],
                                 wv_t[:, c, 0:HLF],
                                 start=(c == 0), stop=(c == CT - 1))
            nc.vector.tensor_add(vaug[:, jt, 0:HLF], ps0, bv_t[:, 0:HLF])
            ps1 = ps_p.tile([P, HLF], fp32, tag="mm", bufs=2, name=f"v1_{jt}")
            for c in range(CT):
                nc.tensor.matmul(ps1, xt[:, c, jt * P:(jt + 1) * P],
                                 wv_t[:, c, HLF:VW],
                                 start=(c == 0), stop=(c == CT - 1))
            nc.vector.tensor_add(vaug[:, jt, HLF:VW], ps1, bv_t[:, HLF:VW])

        def proj_n(ci, n):
            ps_y = ps_p.tile([P, TCH], fp32, tag="mm", bufs=2, name=f"py{ci}_{n}")
            for m in range(4):
                nc.tensor.matmul(ps_y, pw_t[:, m, n * P:(n + 1) * P],
                                 ot[:, m, ci * TCH:(ci + 1) * TCH],
                                 start=(m == 0), stop=(m == 3))
            ys = ys_p.tile([P, TCH], bf16, tag="ys", name=f"ys{ci}_{n}")
            nc.vector.tensor_copy(ys, ps_y)
            nc.sync.dma_start(out=yT_d[n * P:(n + 1) * P,
                                       ci * TCH:(ci + 1) * TCH], in_=ys)

        # ---- attention machinery ----
        pend = []       # queue of pending P@V jobs: (h, ci, ps_o, pts)

        def emit_pv_mm(job, i):
            h, ci, ps_o, pts = job
            jt, pt, off, lo = pts[i]
            nc.tensor.matmul(ps_o[0:65, lo:TCH],
                             vaug[:, jt, h * 65:h * 65 + 65],
                             pt[:, off + lo:off + TCH],
                             start=(i == 0), stop=(i == len(pts) - 1),
                             skip_group_check=True)

        def finish_pv(job):
            h, ci, ps_o, pts = job
            pr, sub = h // 2, h % 2
            # O^T rows (psum 0..63) + denominator row (64) in one DVE copy
            stage = st_p.tile([65, TCH], bf16, tag="st", name=f"st{h}_{ci}")
            nc.vector.tensor_copy(stage, ps_o[0:65, :])
            nc.sync.dma_start(
                out=ot[sub * 64:sub * 64 + 64, pr, ci * TCH:(ci + 1) * TCH],
                in_=stage[0:64, :])
            if ci == NCH - 1:
                # latency-critical tail: fast approx recip of the bf16 den
                # row + rank-1 ones matmul broadcast; no DMA hops
                rcq = rc_p.tile([65, TCH], bf16, tag="rcq", bufs=4,
                                name=f"rcq{h}_{ci}")
                with nc.allow_low_precision(reason="softmax denom recip"):
                    nc.vector.reciprocal(rcq[64:65, :], ps_o[64:65, :])
                rr_tiles[h] = rcq
                if sub == 1:
                    rr = ps_p.tile([P, TCH], fp32, tag="mm", bufs=2,
                                   name=f"rrp{pr}_{ci}")
                    for s2 in range(2):
                        nc.tensor.matmul(rr[s2 * 64:s2 * 64 + 64, :],
                                         ones_t[64:65, :],
                                         rr_tiles[h - 1 + s2][64:65, :],
                                         start=True, stop=True,
                                         skip_group_check=True)
                    nc.vector.tensor_mul(ot[:, pr, ci * TCH:(ci + 1) * TCH],
                                         ot[:, pr, ci * TCH:(ci + 1) * TCH],
                                         rr)
                return
            nc.sync.dma_start(out=den_dram[h:h + 1, ci * TCH:(ci + 1) * TCH],
                              in_=stage[64:65, :])
            if sub == 1:
                # repack this pair's denominators to [128, 8] (DVE recip is
                # free-size-driven), recip, write back, broadcast. In the
                # last chunk the four DMA issues would serialize on one
                # queue right at the tail -- spread them sync/gpsimd there.
                e_gather = nc.gpsimd
                e_wb = nc.gpsimd
                e_bc = [nc.sync, nc.gpsimd]
                chunk = slice(ci * TCH, (ci + 1) * TCH)
                dt = rc_p.tile([P, 8], bf16, tag="dt", name=f"dt{pr}_{ci}")
                e_gather.dma_start(
                    out=dt,
                    in_=den_dram[2 * pr:2 * pr + 2, chunk].rearrange(
                        "h (a f) -> h a f", f=8))
                with nc.allow_low_precision(reason="softmax denom recip"):
                    nc.vector.reciprocal(dt, dt)
                e_wb.dma_start(
                    out=den2_dram[2 * pr:2 * pr + 2, chunk].rearrange(
                        "h (a f) -> h a f", f=8),
                    in_=dt)
                rrf = rr_p.tile([P, TCH], bf16, tag="rrf", name=f"rrf{pr}_{ci}")
                for s2 in range(2):
                    e_bc[s2].dma_start(
                        out=rrf[s2 * 64:s2 * 64 + 64, :],
                        in_=bcast_part(
                            den2_dram[2 * pr + s2:2 * pr + s2 + 1, chunk], 64))
                nc.vector.tensor_mul(ot[:, pr, ci * TCH:(ci + 1) * TCH],
                                     ot[:, pr, ci * TCH:(ci + 1) * TCH], rrf)

        def pair_list(ci):
            prs = [(jt, jt + 1) for jt in range(0, 4 * ci, 2)]
            prs.append((4 * ci + 2, 4 * ci))
            prs.append((4 * ci + 3, 4 * ci + 1))
            return prs

        def s_group(h, ci, lag=2):
            pr, sub = h // 2, h % 2
            rows = slice(sub * 64, sub * 64 + 64)
            qt, kt = qts[pr], kts[pr]
            pairs = pair_list(ci)
            pv_job = pend.pop(0) if len(pend) >= lag else None
            npv = len(pv_job[3]) if pv_job else 0
            pts = []
            ps_o = ps_p.tile([P, TCH], fp32, tag="acc", bufs=2,
                             name=f"acc{h}_{ci}")
            pvi = 0
            for p, (jta, jtb) in enumerate(pairs):
                ps_sp = ps_p.tile([P, 2 * TCH], fp32, tag="sp", bufs=2,
                                  name=f"sp{h}_{ci}_{p}")
                pt_pair = pt_p.tile([P, 2 * TCH], bf16, tag="pt")
                los = []
                # both S matmuls back to back so the exp can issue asap
                for t, jt in enumerate((jta, jtb)):
                    r = jt - 4 * ci
                    lo = 128 * r if r >= 0 else 0
                    off = TCH * t
                    nc.tensor.matmul(ps_sp[:, off + lo:off + TCH],
                                     kt[rows, jt * P:(jt + 1) * P],
                                     qt[rows, ci * TCH + lo:(ci + 1) * TCH],
                                     start=True, stop=True,
                                     skip_group_check=True)
                    pts.append((jt, pt_pair, off, lo))
                    los.append((r, lo, off))
                lo0 = los[0][1]
                nc.scalar.activation(pt_pair[:, lo0:2 * TCH],
                                     ps_sp[:, lo0:2 * TCH],
                                     AF.Exp, scale=float(D) ** -0.5 /
                                     (WS * WS if QK_FP8 else 1.0))
                for r, lo, off in los:
                    if r >= 0:
                        nc.vector.tensor_mul(pt_pair[:, off + lo:off + lo + 128],
                                             pt_pair[:, off + lo:off + lo + 128],
                                             tri_t)
                # interleave pending P@V matmuls after the pair's exp
                tgt = (npv * (p + 1)) // len(pairs)
                while pvi < tgt:
                    emit_pv_mm(pv_job, pvi)
                    pvi += 1
            while pvi < npv:
                emit_pv_mm(pv_job, pvi)
                pvi += 1
            if pv_job is not None:
                finish_pv(pv_job)
            pend.append((h, ci, ps_o, pts))

        def flush_one():
            job = pend.pop(0)
            for i in range(len(job[3])):
                emit_pv_mm(job, i)
            finish_pv(job)

        # ---- fill-work schedule: (earliest_group, thunk) ----
        fills = []
        for i in range(4):
            fills.append((2 * i, lambda pr=i: qk_chunk(pr, 2)))
            fills.append((2 * i + 1, lambda jt=4 + i: vaug_jt(jt)))
            fills.append((8 + 2 * i, lambda pr=i: qk_chunk(pr, 3)))
            fills.append((9 + 2 * i, lambda jt=8 + i: vaug_jt(jt)))
            fills.append((16 + 2 * i, lambda jt=12 + i: vaug_jt(jt)))
        # all proj deferred to ci=3 (its ACT-bound groups need PE filler)
        for n in range(8):
            fills.append((24 + n, lambda n=n: proj_n(0, n)))
            fills.append((25 + (3 * n) // 4, lambda n=n: proj_n(1, n)))
            fills.append((27 + n // 2, lambda n=n: proj_n(2, n)))
        fills.append((26, lambda: flush_one()))
        fills.sort(key=lambda f: f[0])

        # ---- stage A: QK chunks 0-1 + vaug j-tiles 0-3 ----
        for pr in range(4):
            q_half(pr, 0, wq_t, qts[pr], bq_t)
        for pr in range(4):
            q_half(pr, 0, wk_t, kts[pr], bk_t)
        for pr in range(4):
            q_half(pr, 1, wq_t, qts[pr], bq_t)
        for pr in range(4):
            q_half(pr, 1, wk_t, kts[pr], bk_t)
        for jt in range(4):
            vaug_jt(jt)

        # ---- attention groups, ci-major ----
        g = 0
        fi = 0
        for ci in range(NCH):
            for pr in range(4):
                for sub in range(2):
                    while fi < len(fills) and fills[fi][0] <= g:
                        fills[fi][1]()
                        fi += 1
                    lag = 1 if ci == NCH - 1 else 2
                    s_group(2 * pr + sub, ci, lag=lag)
                    g += 1
        while fi < len(fills):
            fills[fi][1]()
            fi += 1
        while pend:
            flush_one()
        for n in range(8):
            proj_n(3, n)
    if not nc.is_finalized():
        nc.finalize()
    return nc


def _prep_inputs(x, qkv_w, qkv_b, proj_w):
    bf = ml_dtypes.bfloat16
    f8 = ml_dtypes.float8_e4m3

    def dr_pack(a):
        # [1024, F] -> [128, 4, 2, F]: c = c2*256 + t*128 + p
        return np.ascontiguousarray(
            a.astype(f8).reshape(4, 2, P, a.shape[1]).transpose(2, 0, 1, 3))

    per_core = []
    wq, wk, wv = qkv_w[0:C], qkv_w[C:2 * C], qkv_w[2 * C:3 * C]
    bq, bk, bv = qkv_b[0:C], qkv_b[C:2 * C], qkv_b[2 * C:3 * C]
    jj = np.arange(P)[:, None]
    ii = np.arange(P)[None, :]
    tri = (jj <= ii).astype(bf)
    xTs = [np.ascontiguousarray(x[b].T).astype(bf) for b in range(B)]
    if QK_FP8:
        xhs = [dr_pack(np.ascontiguousarray(x[b].T)) for b in range(B)]
    for b in range(B):
        for g in range(2):
            hs = slice(g * 512, (g + 1) * 512)
            wvT_aug = np.zeros((C, VW), np.float32)
            bv_aug = np.zeros((1, VW), np.float32)
            for h in range(HPC):
                wvT_aug[:, h * 65:h * 65 + 64] = wv[hs][h * 64:(h + 1) * 64].T
                bv_aug[0, h * 65:h * 65 + 64] = bv[hs][h * 64:(h + 1) * 64]
                bv_aug[0, h * 65 + 64] = 1.0
            core = {
                "xT": xTs[b],
                "wvT": wvT_aug.astype(bf),
                "bv": bv_aug.astype(bf),
                "tri": tri,
                "pwT": np.ascontiguousarray(proj_w[:, hs].T).astype(bf),
            }
            if QK_FP8:
                core.update({
                    "xh": xhs[b],
                    "wqh": dr_pack(WS * wq[hs].T),
                    "wkh": dr_pack(WS * wk[hs].T),
                    "bq": (WS * bq[hs]).reshape(512, 1).astype(np.float32),
                    "bk": (WS * bk[hs]).reshape(512, 1).astype(np.float32),
                })
            else:
                core.update({
                    "wqT": np.ascontiguousarray(wq[hs].T).astype(bf),
                    "wkT": np.ascontiguousarray(wk[hs].T).astype(bf),
                    "bq": bq[hs].reshape(512, 1).astype(np.float32),
                    "bk": bk[hs].reshape(512, 1).astype(np.float32),
                })
            per_core.append(core)
    return per_core


def kernel(x, qkv_w, qkv_b, proj_w, proj_b, _trace=False):
    from concourse.bass_utils import run_bass_kernel_spmd

    x = np.asarray(x, np.float32)
    qkv_w = np.asarray(qkv_w, np.float32)
    qkv_b = np.asarray(qkv_b, np.float32)
    proj_w = np.asarray(proj_w, np.float32)
    proj_b = np.asarray(proj_b, np.float32)

    if "nc" not in _CACHE:
        _CACHE["nc"] = _build_nc()
    nc = _CACHE["nc"]
    in_maps = _prep_inputs(x, qkv_w, qkv_b, proj_w)
    res = run_bass_kernel_spmd(nc, in_maps, core_ids=list(range(8)),
                               trace=_trace)
    _CACHE["last_result"] = res
    y = np.empty((B, T, C), np.float32)
    for b in range(B):
        acc = (res.results[2 * b]["yT"].astype(np.float32)
               + res.results[2 * b + 1]["yT"].astype(np.float32))
        y[b] = acc.T + proj_b
    return y
